# revision 1
# baseline (speedup 1.0000x reference)
"""Trainium2 Bass kernel for nn_AttentionLayer (B=64, S=2048, H=1024).

Computation (per batch b):
    c[b]      = hidden[b] @ W0_hid + b0                      # (H,)  tiny
    z[b,s]    = enc[b,s] @ W0_enc + c[b]                     # main matmul
    score[b,s]= w1 . tanh(z[b,s])        (+ b1, dropped: softmax shift-inv)
    attn      = softmax(where(mask, score, -inf))
    out[b]    = sum_s attn[b,s] * enc[b,s]

Sharding: pure data parallel, 8 batches per core on 8 cores. Params
replicated. Mask handling: masked rows get attention weight exactly 0, so
they are skipped entirely (COMPACT mode): the kernel gathers only the
unmasked rows of enc (indirect DMA by host-computed indices, padded to a
multiple of 128 with bias -1e30 so pads contribute exp(.)=0). This cuts
the dense work by the mask density (~47%) with bit-identical math.

Per s-tile (<=4 chunks of 128 rows):
  - enc rows gathered natural [s,h] (bf16); encT [h,s] produced by the
    hardware DMA-transpose (xbar), from DRAM in dense mode or from SBUF
    in compact mode.
  - z^T[mc] = sum_kc W0e[kc,mc]^T @ encT[kc]   (bf16 matmul, fp32 PSUM)
  - ACT: th = tanh(z^T + bias[mc,b])  fused per-partition bias (c[b]+b0)
  - score psum[1,T] += w1[mc]^T @ th           (accumulating matmul)
  - DVE adds mask bias while copying scores PSUM->SBUF
  - Online softmax (flash-style): running -max, sum l, output o[1,H]
    rescaled by alpha=exp(m_old-m_new); p transposed to [s,1] chunks on
    the PE; contribution matmul p_chunk^T @ enc_chunk accumulated in
    PSUM, folded into o on DVE. Final out = o / l.
  Software pipelining: tile i's softmax/output stage is emitted after
  tile i+1's scores stage so the PE never waits on the DVE/ACT chain.

MODE "f32r" (fallback): full fp32 data, f32r matmuls, PE transposes.
"""

import os
import sys

import numpy as np

for _p in ("/opt/trn_rl_repo", "/root/.axon_site/_ro/trn_rl_repo"):
    if os.path.isdir(_p) and _p not in sys.path:
        sys.path.insert(0, _p)

B, S, H = 64, 2048, 1024
N_CORES = 8
BL = B // N_CORES  # 8 batches per core
NKC = H // 128     # 8 contraction chunks
NMC = H // 128     # 8 output chunks

MODE = "bf16"      # "bf16" (fast, DMA-transpose) or "f32r" (full fp32)
COMPACT = True     # skip masked rows via indirect gather

_CACHE = {}

# Build-time experiment flags (timeline-sim A/B only; leave empty for real runs)
OPTS = {}


def _tile_plan(nch):
    """Split nch 128-row chunks into balanced tiles of <=4 chunks."""
    nt = (nch + 3) // 4
    base, rem = divmod(nch, nt)
    sizes = [base + (1 if i < rem else 0) for i in range(nt)]
    plan, off = [], 0
    for sz in sizes:
        plan.append((off, sz))
        off += sz
    return plan


def _legalize_sync(nc):
    """Strip same-engine semaphore waits (redundant on in-order queues:
    the engine's own monotonic sem already passed the threshold by program
    order). The pinned walrus rejects instructions with >1 sync wait; this
    brings nearly all compute instructions within the limit."""
    from concourse import mybir
    ET = mybir.EngineType
    own = {ET.PE: "PE_", ET.DVE: "DVE_", ET.Activation: "Activation_",
           ET.SP: "SP_", ET.Pool: "Pool_"}
    leftover = []
    for bb in nc.m.functions[0].blocks:
        for ins in bb.instructions:
            si = ins.sync_info
            if si is None or not si.on_wait:
                continue
            pref = own.get(ins.engine)
            w = list(si.on_wait)
            if pref is not None:
                kept = [x for x in w
                        if not (getattr(x, "ant_name", "") or "").startswith(pref)]
                if len(kept) != len(w):
                    si.on_wait = kept
                w = kept
            if len(w) >= 2 and "DMA" not in (ins.opcode or ""):
                leftover.append((ins.name, str(ins.engine), ins.opcode,
                                 [getattr(x, "ant_name", "?") for x in w],
                                 ins.debug.lineno if ins.debug else 0))
    return leftover


def _build(mode, sp, compact):
    import concourse.bass as bass
    import concourse.bacc as bacc
    import concourse.tile as tile
    from concourse import mybir

    F32 = mybir.dt.float32
    F32R = mybir.dt.float32r
    BF16 = mybir.dt.bfloat16
    I32 = mybir.dt.int32
    AF = mybir.ActivationFunctionType
    AX = mybir.AxisListType
    ALU = mybir.AluOpType

    bf = mode == "bf16"
    EDT = BF16 if bf else F32     # enc/params storage dtype
    assert bf or not compact, "compact requires bf16 mode"

    nch = sp // 128               # gathered 128-row chunks per batch
    plan = _tile_plan(nch)

    nc = bacc.Bacc(trn_type="TRN2")

    enc_d = nc.dram_tensor("enc", [BL, S, H], EDT, kind="ExternalInput")
    hid_d = nc.dram_tensor("hid", [BL, H], EDT, kind="ExternalInput")
    mb_d = nc.dram_tensor("mbias", [BL, sp], F32, kind="ExternalInput")
    w0e_d = nc.dram_tensor("W0e", [H, H], EDT, kind="ExternalInput")
    w0h_d = nc.dram_tensor("W0h", [H, H], EDT, kind="ExternalInput")
    b0_d = nc.dram_tensor("b0", [H], F32, kind="ExternalInput")
    w1_d = nc.dram_tensor("w1", [H], EDT, kind="ExternalInput")
    id_d = nc.dram_tensor("ident", [128, 128], EDT, kind="ExternalInput")
    idf_d = nc.dram_tensor("identf", [1, 1], F32, kind="ExternalInput")
    if compact:
        idx_d = nc.dram_tensor("idx", [128, BL * nch], I32,
                               kind="ExternalInput")
    out_d = nc.dram_tensor("out", [BL, H], F32, kind="ExternalOutput")

    def r(ap):
        # matmul-compute view of a stored tile
        return ap if bf else ap.bitcast(F32R)

    with tile.TileContext(nc) as tc:
        from contextlib import ExitStack

        with ExitStack() as ctx:
            persist = ctx.enter_context(tc.tile_pool(name="persist", bufs=1))
            pzp = ctx.enter_context(
                tc.tile_pool(name="pz", bufs=4, space=bass.MemorySpace.PSUM))
            ptrp = ctx.enter_context(
                tc.tile_pool(name="ptr", bufs=1, space=bass.MemorySpace.PSUM))
            pscp = ctx.enter_context(
                tc.tile_pool(name="psc", bufs=1, space=bass.MemorySpace.PSUM))
            pcon = ctx.enter_context(
                tc.tile_pool(name="pcon", bufs=1, space=bass.MemorySpace.PSUM))

            ident = persist.tile([128, 128], EDT, tag="ident")
            nc.gpsimd.dma_start(ident[:], id_d[:])
            identf = persist.tile([1, 1], F32, tag="identf")
            nc.gpsimd.dma_start(identf[:], idf_d[:])
            w0e = persist.tile([128, NKC, H], EDT, tag="w0e")
            nc.sync.dma_start(w0e[:], w0e_d[:].rearrange("(kc p) m -> p kc m", p=128))
            w1s = persist.tile([128, NMC], EDT, tag="w1s")
            nc.gpsimd.dma_start(w1s[:], w1_d[:].rearrange("(mc p) -> p mc", p=128))
            b0s = persist.tile([128, NMC], F32, tag="b0s")
            nc.gpsimd.dma_start(b0s[:], b0_d[:].rearrange("(mc p) -> p mc", p=128))
            biasm = persist.tile([128, NMC * BL], F32, tag="biasm")
            hidT = persist.tile([128, NKC * BL], EDT, tag="hidT")
            if compact:
                idxs = persist.tile([128, BL * nch], I32, tag="idxs")
                nc.gpsimd.dma_start(idxs[:], idx_d[:])
                encflat = enc_d[:].rearrange("b s h -> (b s) h")

            # ---- preamble: bias[mc, b] = (hid[b] @ W0h + b0) per m-chunk ----
            with tc.tile_pool(name="pre", bufs=1) as prep:
                w0h_raw = prep.tile([128, NKC, H], EDT, tag="w0h_raw")
                nc.sync.dma_start(
                    w0h_raw[:], w0h_d[:].rearrange("(kc p) m -> p kc m", p=128))
                w0h = prep.tile([128, NKC, H], EDT, tag="w0h")
                nc.vector.tensor_copy(w0h[:], w0h_raw[:])
                hids_raw = prep.tile([BL, H], EDT, tag="hids_raw")
                nc.gpsimd.dma_start(hids_raw[:], hid_d[:])
                hids = prep.tile([BL, H], EDT, tag="hids")
                nc.vector.tensor_copy(hids[:], hids_raw[:])
                idv = prep.tile([128, 128], EDT, tag="idv")
                nc.vector.tensor_copy(idv[:], ident[:])
                b0f = prep.tile([128, NMC], F32, tag="b0f")
                nc.scalar.copy(b0f[:], b0s[:])

                ptr = ptrp.tile([128, NKC * BL], EDT, tag="trp")
                for kc in range(NKC):
                    nc.tensor.transpose(
                        r(ptr[:, kc * BL:(kc + 1) * BL]),
                        r(hids[0:BL, kc * 128:(kc + 1) * 128]),
                        r(idv[0:BL, 0:BL]))
                nc.vector.tensor_copy(hidT[:], ptr[:])

                for mc in range(NMC):
                    pz = pzp.tile([128, 512], F32, tag="pz")
                    for kc in range(NKC):
                        nc.tensor.matmul(
                            pz[:, 0:BL],
                            r(w0h[:, kc, mc * 128:(mc + 1) * 128]),
                            r(hidT[:, kc * BL:(kc + 1) * BL]),
                            start=(kc == 0), stop=(kc == NKC - 1))
                    nc.scalar.activation(
                        biasm[:, mc * BL:(mc + 1) * BL], pz[:, 0:BL],
                        AF.Identity, bias=b0f[:, mc:mc + 1])

            # ---- main pools ----
            encp = ctx.enter_context(tc.tile_pool(name="encp", bufs=4))
            encTp = ctx.enter_context(tc.tile_pool(name="encT", bufs=2))
            thp = ctx.enter_context(tc.tile_pool(name="th", bufs=2))
            scp = ctx.enter_context(tc.tile_pool(name="sc", bufs=2))
            mbp = ctx.enter_context(tc.tile_pool(name="mb", bufs=2))
            ptsp = ctx.enter_context(tc.tile_pool(name="pts", bufs=2))
            smp = ctx.enter_context(tc.tile_pool(name="sm", bufs=2))
            obp = ctx.enter_context(tc.tile_pool(name="ob", bufs=2))
            outp = ctx.enter_context(tc.tile_pool(name="outp", bufs=2))

            def stage_scores(b, ti, bst):
                """One s-tile: scores into SBUF (main matmul + tanh + w1)."""
                c0, ncs = plan[ti]
                stt = ncs * 128
                enc_nat = encp.tile([128, 4, H], EDT, tag="enc")
                encT = encTp.tile([128, NKC, 512], EDT, tag="encT")
                if compact:
                    for j in range(ncs):
                        nc.gpsimd.indirect_dma_start(
                            out=enc_nat[:, j, :],
                            out_offset=None,
                            in_=encflat,
                            in_offset=bass.IndirectOffsetOnAxis(
                                ap=idxs[:, b * nch + c0 + j:
                                        b * nch + c0 + j + 1],
                                axis=0))
                    for j in range(ncs):
                        nc.sync.dma_start_transpose(
                            encT[:, :, j * 128:(j + 1) * 128],
                            enc_nat[:, j, :])
                else:
                    nc.sync.dma_start(
                        enc_nat[:, 0:ncs, :],
                        enc_d[b, c0 * 128:c0 * 128 + stt, :]
                        .rearrange("(ss p) h -> p ss h", p=128))
                    if bf:
                        for j in range(ncs):
                            s0 = (c0 + j) * 128
                            nc.sync.dma_start_transpose(
                                encT[:, :, j * 128:(j + 1) * 128],
                                enc_d[b, s0:s0 + 128, :])
                    else:
                        for kc in range(NKC):
                            ptr = ptrp.tile([128, 512], F32, tag="tr")
                            for ss in range(ncs):
                                nc.tensor.transpose(
                                    r(ptr[:, ss * 128:(ss + 1) * 128]),
                                    r(enc_nat[:, ss,
                                              kc * 128:(kc + 1) * 128]),
                                    r(ident[:]))
                            nc.vector.tensor_copy(
                                encT[:, kc, 0:stt], ptr[:, 0:stt])

                psc = pscp.tile([1, 512], F32, tag="psc")
                for mc in range(NMC):
                    pz = pzp.tile([128, 512], F32, tag="pz")
                    if OPTS.get("no_main"):
                        nc.tensor.matmul(
                            pz[0:1, 0:stt], r(ident[0:1, 0:1]),
                            r(encT[0:1, 0, 0:stt]), start=True, stop=True)
                    else:
                        for kc in range(NKC):
                            nc.tensor.matmul(
                                pz[:, 0:stt],
                                r(w0e[:, kc, mc * 128:(mc + 1) * 128]),
                                r(encT[:, kc, 0:stt]),
                                start=(kc == 0), stop=(kc == NKC - 1))
                    th = thp.tile([128, 512], EDT, tag="th")
                    nc.scalar.activation(
                        th[:, 0:stt], pz[:, 0:stt], AF.Tanh,
                        bias=biasm[:, mc * BL + b:mc * BL + b + 1])
                    nc.tensor.matmul(
                        psc[:, 0:stt], r(w1s[:, mc:mc + 1]), r(th[:, 0:stt]),
                        start=(mc == 0), stop=(mc == NMC - 1))

                # scores tile -> SBUF, then mask bias add
                sc_sb = scp.tile([1, 512], F32, tag="sc")
                nc.vector.tensor_copy(sc_sb[:, 0:stt], psc[:, 0:stt])
                nc.vector.tensor_add(
                    sc_sb[:, 0:stt], sc_sb[:, 0:stt],
                    bst["mb"][:, c0 * 128:c0 * 128 + stt])
                return sc_sb, enc_nat

            def stage_update(b, ti, bst, sc_sb, enc_nat):
                """Online softmax + output accumulation for one s-tile."""
                c0, ncs = plan[ti]
                stt = ncs * 128
                nm_run, l_run, o_sb = bst["nm"], bst["l"], bst["o"]
                tmp = smp.tile([1, 4], F32, tag="tmp")
                nm_t, alpha, sum_p = tmp[:, 0:1], tmp[:, 1:2], tmp[:, 2:3]
                nc.vector.reduce_max(
                    nm_t, sc_sb[:, 0:stt], axis=AX.X, negate=True)
                if ti == 0:
                    nc.vector.tensor_copy(nm_run[:], nm_t)
                    # p = exp(score - m); l = sum(p)
                    nc.scalar.activation(
                        sc_sb[:, 0:stt], sc_sb[:, 0:stt], AF.Exp,
                        bias=nm_run[:], accum_out=l_run[:])
                else:
                    # nm_new = min(nm_run, nm_t) = -max(m_run, m_t)
                    nc.vector.tensor_tensor(
                        out=nm_t, in0=nm_t, in1=nm_run[:], op=ALU.min)
                    # alpha = exp(m_old - m_new) = exp(nm_new - nm_old)
                    nc.vector.tensor_sub(alpha, nm_t, nm_run[:])
                    nc.scalar.activation(alpha, alpha, AF.Exp)
                    nc.vector.tensor_copy(nm_run[:], nm_t)
                    nc.scalar.activation(
                        sc_sb[:, 0:stt], sc_sb[:, 0:stt], AF.Exp,
                        bias=nm_run[:], accum_out=sum_p)
                    # l = l*alpha + sum_p
                    nc.vector.tensor_tensor(
                        out=l_run[:], in0=l_run[:], in1=alpha, op=ALU.mult)
                    nc.vector.tensor_add(l_run[:], l_run[:], sum_p)

                # transpose p [1,stt] -> pT [128, ncs] chunks (fp32 PE, tiny)
                ptr = ptrp.tile([128, 4], F32, tag="trp")
                for ss in range(ncs):
                    nc.tensor.transpose(
                        ptr[:, ss:ss + 1].bitcast(F32),
                        sc_sb[0:1, ss * 128:(ss + 1) * 128].bitcast(F32),
                        identf[:])
                pT = ptsp.tile([128, 4], EDT, tag="pT")
                nc.vector.tensor_copy(pT[:, 0:ncs], ptr[:, 0:ncs])

                # contribution: sum_s p[s] * enc[s, :]
                pc = pcon.tile([1, H], F32, tag="pc")
                for ss in range(ncs):
                    for nh in range(2):
                        nc.tensor.matmul(
                            pc[:, nh * 512:(nh + 1) * 512],
                            r(pT[:, ss:ss + 1]),
                            r(enc_nat[:, ss, nh * 512:(nh + 1) * 512]),
                            start=(ss == 0), stop=(ss == ncs - 1))
                if ti == 0:
                    nc.vector.tensor_copy(o_sb[:], pc[:])
                else:
                    # o = o*alpha + contrib
                    nc.vector.tensor_tensor(
                        out=o_sb[:], in0=o_sb[:],
                        in1=alpha.to_broadcast([1, H]), op=ALU.mult)
                    nc.vector.tensor_add(o_sb[:], o_sb[:], pc[:])

            def finish_batch(b, bst):
                tmp = smp.tile([1, 4], F32, tag="tmp")
                linv = tmp[:, 3:4]
                nc.vector.reciprocal(linv, bst["l"][:])
                outt = outp.tile([1, H], F32, tag="out")
                nc.vector.tensor_tensor(
                    out=outt[:], in0=bst["o"][:],
                    in1=linv.to_broadcast([1, H]), op=ALU.mult)
                nc.gpsimd.dma_start(out_d[b:b + 1, :], outt[:])

            pending = None  # (b, ti, bst, sc_sb, enc_nat) one s-tile behind
            for b in range(BL):
                mbt = mbp.tile([1, sp], F32, tag="mbt")
                nc.gpsimd.dma_start(mbt[:], mb_d[b:b + 1, :])
                nm_tile = smp.tile([1, 1], F32, tag="nm")
                l_tile = smp.tile([1, 1], F32, tag="l")
                o_tile = obp.tile([1, H], F32, tag="o")
                bst = {"mb": mbt, "nm": nm_tile, "l": l_tile, "o": o_tile}
                for ti in range(len(plan)):
                    sc_sb, enc_nat = stage_scores(b, ti, bst)
                    if pending is not None:
                        pb, pti, pbst, psb, pen = pending
                        stage_update(pb, pti, pbst, psb, pen)
                        if pti == len(plan) - 1:
                            finish_batch(pb, pbst)
                    pending = (b, ti, bst, sc_sb, enc_nat)
            pb, pti, pbst, psb, pen = pending
            stage_update(pb, pti, pbst, psb, pen)
            finish_batch(pb, pbst)

    nc.compile()  # bacc lowering: splits waits to <=1 per instruction
    return nc


def _get_nc(sp):
    key = (MODE, COMPACT, sp)
    if key not in _CACHE:
        _CACHE[key] = _build(MODE, sp, COMPACT)
    return _CACHE[key]


def _prep(hidden, enc_seq, mask, W0, b0, w1):
    import ml_dtypes
    bf = MODE == "bf16"
    edt = ml_dtypes.bfloat16 if bf else np.float32

    mask = np.asarray(mask).astype(bool)
    enc = np.ascontiguousarray(np.asarray(enc_seq).astype(edt))
    hid = np.ascontiguousarray(np.asarray(hidden).reshape(B, H).astype(edt))
    W0 = np.asarray(W0, dtype=np.float32)
    w0e = np.ascontiguousarray(W0[:H].astype(edt))
    w0h = np.ascontiguousarray(W0[H:].astype(edt))
    b0 = np.ascontiguousarray(np.asarray(b0, dtype=np.float32))
    w1 = np.ascontiguousarray(np.asarray(w1).astype(edt))
    ident = np.eye(128, dtype=np.float32).astype(edt)
    identf = np.ones((1, 1), dtype=np.float32)

    if COMPACT:
        counts = mask.sum(axis=1)
        sp = max(128, int(-(-counts.max() // 128)) * 128)
        nch = sp // 128
    else:
        sp = S
        nch = S // 128

    maps = []
    for c in range(N_CORES):
        sl = slice(c * BL, (c + 1) * BL)
        m = {"enc": enc[sl], "hid": hid[sl],
             "W0e": w0e, "W0h": w0h, "b0": b0, "w1": w1,
             "ident": ident, "identf": identf}
        if COMPACT:
            mbc = np.full((BL, sp), -1e30, dtype=np.float32)
            idxT = np.zeros((128, BL * nch), dtype=np.int32)
            for b in range(BL):
                rows = np.flatnonzero(mask[c * BL + b])
                cnt = len(rows)
                mbc[b, :cnt] = 0.0
                flat = np.zeros(sp, dtype=np.int32)
                flat[:cnt] = b * S + rows
                idxT[:, b * nch:(b + 1) * nch] = (
                    flat.reshape(nch, 128).T)
            m["mbias"] = mbc
            m["idx"] = idxT
        else:
            m["mbias"] = np.where(mask[sl], np.float32(0.0),
                                  np.float32(-1e30)).astype(np.float32)
        maps.append(m)
    return maps, sp


def _run(in_maps, sp, **kwargs):
    from concourse.bass_utils import run_bass_kernel_spmd
    nc = _get_nc(sp)
    res = run_bass_kernel_spmd(nc, in_maps, list(range(N_CORES)), **kwargs)
    out = np.concatenate(
        [res.results[c]["out"] for c in range(N_CORES)], axis=0)
    return out, res


def kernel(hidden, enc_seq, mask, W0, b0, w1, b1):
    # b1 shifts every score equally -> cancelled by softmax; unused.
    in_maps, sp = _prep(hidden, enc_seq, mask, W0, b0, w1)
    out, _ = _run(in_maps, sp)
    return out


def kernel_profiled(hidden, enc_seq, mask, W0, b0, w1, b1, **kwargs):
    in_maps, sp = _prep(hidden, enc_seq, mask, W0, b0, w1)
    out, res = _run(in_maps, sp, trace=True, **kwargs)
    return out, res

